# revision 8
# baseline (speedup 1.0000x reference)
"""Bass/Trainium2 kernel for a GPT-2-style transformer block (B=4, T=2048,
C=768, H=12, causal attention + GELU MLP), SPMD across 8 NeuronCores.

Sharding: core i = (batch b = i//2, head-group g = i%2, 6 heads each).
Per core: LN1 + QKV over its batch's 2048 tokens (its 6 heads only),
causal flash-style attention (scores computed transposed [k, q], softmax
denominator via an appended ones-column in V, constant-shift-free exp —
scores are bounded ~|8| for LN'd inputs), attention out-projection
partial sums, pairwise ReduceScatter(add) over {2b, 2b+1} which also
splits the 2048 tokens in half, then residual + LN2 + MLP on the core's
1024 tokens.  All matmuls run in bf16 with fp32 PSUM accumulation.
"""

import sys
import types

import numpy as np
import orjson

B, T, C, H = 4, 2048, 768, 12
HD = C // H          # 64 head dim
HG = H // 2          # 6 heads per core
OC = 3 * HG * HD     # 1152 qkv cols per core
EPS = 1e-5
N_CORES = 8
P = 128
NT = T // P          # 16 token blocks per batch
NC6 = C // P         # 6 c-tiles
TOWN = T // 2        # 1024 own tokens after reduce-scatter
NOB = TOWN // P      # 8 own token blocks
FF = 4 * C           # 3072
NFF = FF // P        # 24
VW = HD + 1          # 65 V cols per head incl the ones column


# ---------------------------------------------------------------------------
# Toolchain patches for this container:
# 1. walrus accepts only ONE sync wait per instruction; Tile emits more.
#    Split extras onto NoOp instructions inserted before the owner (same
#    engine, program order — semantically identical).
# 2. antenv.axon_hooks is absent, so trace=True would fail; register the
#    NTFF hook module ourselves (used by test.py; harmless otherwise).
# ---------------------------------------------------------------------------

def _install_birpatch():
    import concourse.bass as bass

    if getattr(bass.Bass.to_json_bytes, "_multiwait_patched", False):
        return
    orig = bass.Bass.to_json_bytes

    def split_multi_waits(bir):
        ctr = 0
        for fn in bir.get("functions", []):
            for blk in fn.get("blocks", []):
                out = []
                for inst in blk.get("instructions", []):
                    si = inst.get("sync_info")
                    waits = (si or {}).get("on_wait", [])
                    if len(waits) > 1:
                        for w in waits[:-1]:
                            ctr += 1
                            out.append({
                                "debug": inst.get("debug", 0),
                                "engine": inst["engine"],
                                "ins": [],
                                "outs": [],
                                "name": f"waitsplit-{ctr}",
                                "opcode": "NoOp",
                                "sync_info": {"on_update": [],
                                              "on_wait": [w]},
                            })
                        si["on_wait"] = waits[-1:]
                    out.append(inst)
                blk["instructions"] = out
        return bir

    def patched(self):
        return orjson.dumps(split_multi_waits(orjson.loads(orig(self))))

    patched._multiwait_patched = True
    bass.Bass.to_json_bytes = patched


def _install_ntff_hook():
    if "antenv.axon_hooks" in sys.modules:
        return
    try:
        from trn_agent_boot.trn_boot import _ntff_profile_via_ctypes

        hook = _ntff_profile_via_ctypes("/opt/axon/libaxon_pjrt.so")
    except Exception:
        hook = None
    m = types.ModuleType("antenv.axon_hooks")
    m.get_axon_ntff_profile_hook = lambda: hook
    m.set_axon_ntff_profile_hook = lambda h: None
    sys.modules["antenv.axon_hooks"] = m


def _build_program():
    import concourse.bass as bass
    import concourse.mybir as mybir
    from concourse.masks import make_identity
    from concourse.tile import TileContext

    dt = mybir.dt
    AF = mybir.ActivationFunctionType
    ALU = mybir.AluOpType

    nc = bass.Bass("TRN2", target_bir_lowering=False, debug=False,
                   num_devices=N_CORES)

    # ---- I/O -----------------------------------------------------------
    xb = nc.dram_tensor("xb", [T, C], dt.float32, kind="ExternalInput")
    xown = nc.dram_tensor("xown", [TOWN, C], dt.float32, kind="ExternalInput")
    wqkv = nc.dram_tensor("wqkv", [C, OC], dt.float32, kind="ExternalInput")
    bqkv = nc.dram_tensor("bqkv", [OC], dt.float32, kind="ExternalInput")
    wo = nc.dram_tensor("wo", [HG * HD, C], dt.float32, kind="ExternalInput")
    bo = nc.dram_tensor("bo", [C], dt.float32, kind="ExternalInput")
    ln1s = nc.dram_tensor("ln1s", [C], dt.float32, kind="ExternalInput")
    ln1b = nc.dram_tensor("ln1b", [C], dt.float32, kind="ExternalInput")
    ln2s = nc.dram_tensor("ln2s", [C], dt.float32, kind="ExternalInput")
    ln2b = nc.dram_tensor("ln2b", [C], dt.float32, kind="ExternalInput")
    wfc = nc.dram_tensor("wfc", [C, FF], dt.float32, kind="ExternalInput")
    bfc = nc.dram_tensor("bfc", [FF], dt.float32, kind="ExternalInput")
    wpr = nc.dram_tensor("wpr", [FF, C], dt.float32, kind="ExternalInput")
    bpr = nc.dram_tensor("bpr", [C], dt.float32, kind="ExternalInput")
    out = nc.dram_tensor("out", [TOWN, C], dt.float32, kind="ExternalOutput")

    # internal DRAM
    po_dram = nc.dram_tensor("po_dram", [T, C], dt.float32)
    rs_dram = nc.dram_tensor("rs_dram", [TOWN, C], dt.float32)
    r_dram = nc.dram_tensor("r_dram", [HG * 4, 512], dt.float32)
    x2_dram = nc.dram_tensor("x2_dram", [TOWN, C], dt.float32)
    wfcb_dram = nc.dram_tensor("wfcb_dram", [C, FF], dt.bfloat16)
    wprb_dram = nc.dram_tensor("wprb_dram", [FF, C], dt.bfloat16)

    groups = [[0, 1], [2, 3], [4, 5], [6, 7]]

    with TileContext(nc) as tc:
        with (
            tc.tile_pool(name="const", bufs=1) as const,
            tc.tile_pool(name="persist", bufs=1) as persist,
            tc.tile_pool(name="wstage", bufs=2) as wstage,
            tc.tile_pool(name="wsbp", bufs=1) as wsbp,
            tc.tile_pool(name="rbp", bufs=2) as rbp,
            tc.tile_pool(name="rpool", bufs=2) as rpool,
            tc.tile_pool(name="wstream", bufs=4) as wstream,
            tc.tile_pool(name="xtmp", bufs=2) as xtmp,
            tc.tile_pool(name="stat", bufs=4) as statp,
            tc.tile_pool(name="epool", bufs=3) as epool,
            tc.tile_pool(name="evict", bufs=2) as evict,
            tc.tile_pool(name="psmm", bufs=4, space="PSUM") as psmm,
            tc.tile_pool(name="psctx", bufs=2, space="PSUM") as psctx,
            tc.tile_pool(name="pstr", bufs=2, space="PSUM") as pstr,
        ):
            # ---- constants -------------------------------------------
            ident = const.tile([P, P], dt.bfloat16, tag="ident")
            make_identity(nc, ident)
            tri = const.tile([P, 896], dt.bfloat16, tag="tri")
            nc.gpsimd.memset(tri[:], 1.0)
            nc.gpsimd.affine_select(
                out=tri[:], in_=tri[:], compare_op=ALU.is_ge, fill=0.0,
                base=-384, pattern=[[1, 896]], channel_multiplier=-1)

            def rep_row(drt, tag):
                t = const.tile([P, C], dt.float32, tag=tag)
                nc.sync.dma_start(
                    t[:], drt.ap().unsqueeze(0).broadcast_to((P, C)))
                return t

            # early-phase / late-phase pairs share a slot
            s1r = rep_row(ln1s, "rrowA")
            b1r = rep_row(ln1b, "rrowB")
            s2r = rep_row(ln2s, "rrowC")
            b2r = rep_row(ln2b, "rrowD")
            bor = rep_row(bo, "rrowE")
            bprr = rep_row(bpr, "rrowF")
            bqkv9 = const.tile([P, OC // P], dt.float32, tag="bq9")
            nc.sync.dma_start(
                bqkv9[:], bqkv.ap().rearrange("(t p) -> p t", p=P))
            bfc24 = const.tile([P, NFF], dt.float32, tag="bf24")
            nc.sync.dma_start(
                bfc24[:], bfc.ap().rearrange("(t p) -> p t", p=P))
            epsb = const.tile([P, 1], dt.float32, tag="epsb")
            nc.gpsimd.memset(epsb[:], EPS)
            bvr = const.tile([P, HG * HD], dt.float32, tag="bvr")
            nc.sync.dma_start(
                bvr[:],
                bqkv.ap()[2 * HG * HD:].unsqueeze(0)
                .broadcast_to((P, HG * HD)))

            # ---- resident weights: wqkv, wo (bf16) -------------------
            wqkv_b = [persist.tile([P, OC], dt.bfloat16, name=f"wqkv{c}", tag=f"wqkv{c}")
                      for c in range(NC6)]
            for c in range(NC6):
                wt = wstage.tile([P, 1536], dt.float32, tag="wst")
                nc.sync.dma_start(wt[:, :OC], wqkv[c * P:(c + 1) * P, :])
                nc.vector.tensor_copy(wqkv_b[c][:], wt[:, :OC])
            wo_b = [persist.tile([P, C], dt.bfloat16, name=f"wo{d}", tag=f"wo{d}")
                    for d in range(3)]
            for d in range(3):
                wt = wstage.tile([P, 1536], dt.float32, tag="wst")
                nc.sync.dma_start(wt[:, :C], wo[d * P:(d + 1) * P, :])
                nc.vector.tensor_copy(wo_b[d][:], wt[:, :C])

            # ---- pre-cast wfc / wpr to bf16 in DRAM (streamed later) --
            for c in range(NC6):
                for hh in range(2):
                    cols = slice(hh * 1536, (hh + 1) * 1536)
                    wt = wstage.tile([P, 1536], dt.float32, tag="wst")
                    nc.sync.dma_start(wt[:], wfc[c * P:(c + 1) * P, cols])
                    wb = wsbp.tile([P, 1536], dt.bfloat16, tag="wsb")
                    nc.vector.tensor_copy(wb[:], wt[:])
                    nc.sync.dma_start(
                        wfcb_dram[c * P:(c + 1) * P, cols], wb[:])
            for h in range(NFF):
                wt = wstage.tile([P, 1536], dt.float32, tag="wst")
                nc.sync.dma_start(wt[:, :C], wpr[h * P:(h + 1) * P, :])
                wb = wsbp.tile([P, 1536], dt.bfloat16, tag="wsb")
                nc.vector.tensor_copy(wb[:, :C], wt[:, :C])
                nc.sync.dma_start(wprb_dram[h * P:(h + 1) * P, :], wb[:, :C])

            # ---- LN1 + transpose -> xnT ------------------------------
            xnT = [[persist.tile([P, T // 2], dt.bfloat16, name=f"xnT{c}_{th}", tag=f"xnT{c}_{th}")
                    for th in range(2)] for c in range(NC6)]

            def layernorm_tile(xt, sr, br, dst_bf):
                st6 = statp.tile([P, 2, 6], dt.float32, tag="st6")
                st2 = statp.tile([P, 2], dt.float32, tag="st2")
                nc.vector.bn_stats(st6[:, 0, :], xt[:, 0:384])
                nc.vector.bn_stats(st6[:, 1, :], xt[:, 384:768])
                nc.vector.bn_aggr(st2[:], st6[:])
                sd = statp.tile([P, 1], dt.float32, tag="sd")
                nc.scalar.activation(sd[:], st2[:, 1:2], AF.Sqrt, bias=epsb[:])
                rs = statp.tile([P, 1], dt.float32, tag="rs")
                nc.vector.reciprocal(rs[:], sd[:])
                t1 = xtmp.tile([P, C], dt.float32, tag="t1")
                nc.vector.scalar_tensor_tensor(
                    t1[:], xt[:], st2[:, 0:1], sr[:],
                    op0=ALU.subtract, op1=ALU.mult)
                nc.vector.scalar_tensor_tensor(
                    dst_bf[:], t1[:], rs[:], br[:],
                    op0=ALU.mult, op1=ALU.add)

            for ti in range(NT):
                xt = xtmp.tile([P, C], dt.float32, tag="xt")
                nc.sync.dma_start(xt[:], xb[ti * P:(ti + 1) * P, :])
                xn = xtmp.tile([P, C], dt.bfloat16, tag="xn")
                layernorm_tile(xt, s1r, b1r, xn)
                th, tcol = ti // (NT // 2), (ti % (NT // 2)) * P
                for c in range(NC6):
                    pt = pstr.tile([P, P], dt.bfloat16, tag="ptr")
                    nc.tensor.transpose(
                        pt[:], xn[:, c * P:(c + 1) * P], ident[:])
                    nc.vector.tensor_copy(
                        xnT[c][th][:, tcol:tcol + P], pt[:])

            # ---- QKV: qkT [o, t] (Q tiles 0-2, K tiles 3-5) ----------
            qkT = [persist.tile([P, T], dt.bfloat16, name=f"qkT{o}", tag=f"qkT{o}")
                   for o in range(6)]
            vnat = [persist.tile([P, HG * VW], dt.bfloat16, name=f"vnat{tb}", tag=f"vnat{tb}")
                    for tb in range(NT)]
            for th in range(2):
                for tck in range(2):
                    cols = slice(tck * 512, (tck + 1) * 512)
                    gcol = th * 1024 + tck * 512
                    for o in range(6):
                        ps = psmm.tile([P, 512], dt.float32, tag="mm")
                        for c in range(NC6):
                            nc.tensor.matmul(
                                ps[:], wqkv_b[c][:, o * P:(o + 1) * P],
                                xnT[c][th][:, cols],
                                start=(c == 0), stop=(c == NC6 - 1))
                        nc.scalar.activation(
                            qkT[o][:, gcol:gcol + 512], ps[:], AF.Identity,
                            bias=bqkv9[:, o:o + 1])
            # V in natural layout [t, head*65] with a ones column per head
            for tb in range(NT):
                nc.gpsimd.memset(vnat[tb][:], 1.0)
                ps = psmm.tile([P, 512], dt.float32, tag="mm")
                th, tcol = tb // (NT // 2), (tb % (NT // 2)) * P
                for c in range(NC6):
                    nc.tensor.matmul(
                        ps[:, :HG * HD],
                        xnT[c][th][:, tcol:tcol + P],
                        wqkv_b[c][:, 2 * HG * HD:3 * HG * HD],
                        start=(c == 0), stop=(c == NC6 - 1))
                vv = vnat[tb].rearrange("p (h w) -> p h w", w=VW)
                nc.vector.scalar_tensor_tensor(
                    vv[:, :, 0:HD],
                    ps[:, :HG * HD].rearrange("p (h w) -> p h w", w=HD),
                    0.0,
                    bvr.rearrange("p (h w) -> p h w", w=HD),
                    op0=ALU.add, op1=ALU.add)

            # ---- causal attention (scores transposed [k, q]) ---------
            ctxT = [persist.tile([P, T], dt.bfloat16, name=f"ctxT{d}", tag=f"ctxT{d}")
                    for d in range(3)]
            for h in range(HG):
                ot = h // 2
                prow = (h % 2) * HD
                for qc in range(4):
                    qcols = slice(qc * 512, (qc + 1) * 512)
                    nkb = 4 * (qc + 1)
                    ctx = psctx.tile([VW, 512], dt.float32, tag="ctx")
                    for kb in range(nkb):
                        st = psmm.tile([P, 512], dt.float32, tag="mm")
                        nc.tensor.matmul(
                            st[:],
                            qkT[3 + ot][prow:prow + HD, kb * P:(kb + 1) * P],
                            qkT[ot][prow:prow + HD, qcols],
                            start=True, stop=True)
                        e = epool.tile([P, 512], dt.bfloat16, tag="e")
                        nc.scalar.activation(e[:], st[:], AF.Exp, scale=0.125)
                        if kb >= 4 * qc:
                            j0 = 384 + qc * 512 - kb * P
                            nc.vector.tensor_mul(
                                e[:], e[:], tri[:, j0:j0 + 512])
                        nc.tensor.matmul(
                            ctx[:], vnat[kb][:, h * VW:(h + 1) * VW], e[:],
                            start=(kb == 0), stop=(kb == nkb - 1))
                    r = rpool.tile([1, 512], dt.float32, tag="r")
                    nc.vector.reciprocal(r[:], ctx[HD:HD + 1, :])
                    ridx = h * 4 + qc
                    nc.sync.dma_start(r_dram[ridx, :], r[:])
                    rb = rbp.tile([HD, 512], dt.float32, tag="rb")
                    nc.sync.dma_start(
                        rb[:],
                        r_dram.ap()[ridx, :].unsqueeze(0)
                        .broadcast_to((HD, 512)))
                    nc.vector.tensor_mul(
                        ctxT[ot][prow:prow + HD, qcols], ctx[0:HD, :], rb[:])

            # ---- attention out-projection partials -> po_dram --------
            for tb in range(NT):
                po = evict.tile([P, C], dt.float32, tag="po")
                for nh in range(2):
                    ps = psmm.tile([P, 512], dt.float32, tag="mm")
                    for d in range(3):
                        nc.tensor.matmul(
                            ps[:, :384], ctxT[d][:, tb * P:(tb + 1) * P],
                            wo_b[d][:, nh * 384:(nh + 1) * 384],
                            start=(d == 0), stop=(d == 2))
                    nc.scalar.activation(
                        po[:, nh * 384:(nh + 1) * 384], ps[:, :384], AF.Copy)
                nc.sync.dma_start(po_dram[tb * P:(tb + 1) * P, :], po[:])

            # ---- pairwise reduce-scatter over {2b, 2b+1} -------------
            nc.gpsimd.collective_compute(
                "ReduceScatter", mybir.AluOpType.add,
                replica_groups=groups,
                ins=[po_dram[:]], outs=[rs_dram[:]])

            # ---- residual + LN2 + transpose --------------------------
            # xn2T reuses the (now dead) qkT slots
            xn2T = [persist.tile([P, TOWN], dt.bfloat16, name=f"xn2T{c}", tag=f"qkT{c}")
                    for c in range(NC6)]
            for ob in range(NOB):
                rt = xtmp.tile([P, C], dt.float32, tag="xt")
                nc.sync.dma_start(rt[:], rs_dram[ob * P:(ob + 1) * P, :])
                xo = xtmp.tile([P, C], dt.float32, tag="t1")
                nc.sync.dma_start(xo[:], xown[ob * P:(ob + 1) * P, :])
                t0 = xtmp.tile([P, C], dt.float32, tag="t0")
                nc.vector.tensor_add(t0[:], rt[:], bor[:])
                x2t = xtmp.tile([P, C], dt.float32, tag="x2t")
                nc.vector.tensor_add(x2t[:], t0[:], xo[:])
                nc.sync.dma_start(x2_dram[ob * P:(ob + 1) * P, :], x2t[:])
                xn2 = xtmp.tile([P, C], dt.bfloat16, tag="xn")
                layernorm_tile(x2t, s2r, b2r, xn2)
                for c in range(NC6):
                    pt = pstr.tile([P, P], dt.bfloat16, tag="ptr")
                    nc.tensor.transpose(
                        pt[:], xn2[:, c * P:(c + 1) * P], ident[:])
                    nc.vector.tensor_copy(
                        xn2T[c][:, ob * P:(ob + 1) * P], pt[:])

            # ---- MLP (chunks of 512 own tokens) ----------------------
            # hT chunk tiles: first 12 reuse dead xnT slots
            hT = []
            for o in range(NFF):
                if o < 12:
                    tg = f"xnT{o % 6}_{o // 6}"
                else:
                    tg = f"hT{o}"
                hT.append(persist.tile([P, 512], dt.bfloat16, name=f"hT{o}", tag=tg))

            for ck in range(2):
                cols = slice(ck * 512, (ck + 1) * 512)
                # fc + gelu -> hT
                for o in range(NFF):
                    wtiles = []
                    for c in range(NC6):
                        w = wstream.tile([P, P], dt.bfloat16, name=f"wfcs_{ck}_{o}_{c}", tag="wfcs")
                        nc.sync.dma_start(
                            w[:], wfcb_dram[c * P:(c + 1) * P,
                                            o * P:(o + 1) * P])
                        wtiles.append(w)
                    ps = psmm.tile([P, 512], dt.float32, tag="mm")
                    for c in range(NC6):
                        nc.tensor.matmul(
                            ps[:], wtiles[c][:], xn2T[c][:, cols],
                            start=(c == 0), stop=(c == NC6 - 1))
                    nc.scalar.activation(
                        hT[o][:], ps[:], AF.Gelu_apprx_tanh,
                        bias=bfc24[:, o:o + 1])
                # proj2 + bias + residual -> out, two (2 ob x 2 nh) rounds
                for rnd in range(2):
                    obs = [ck * 4 + rnd * 2, ck * 4 + rnd * 2 + 1]
                    pss = [[psmm.tile([P, 512], dt.float32, name=f"pp{ck}_{rnd}_{i}_{j}", tag="mm")
                            for j in range(2)] for i in range(2)]
                    for ht in range(NFF):
                        w = wstream.tile([P, C], dt.bfloat16, name=f"wprs_{ck}_{rnd}_{ht}", tag="wprs")
                        nc.sync.dma_start(
                            w[:], wprb_dram[ht * P:(ht + 1) * P, :])
                        for i in range(2):
                            lcol = (obs[i] - ck * 4) * P
                            for j in range(2):
                                nc.tensor.matmul(
                                    pss[i][j][:, :384],
                                    hT[ht][:, lcol:lcol + P],
                                    w[:, j * 384:(j + 1) * 384],
                                    start=(ht == 0), stop=(ht == NFF - 1))
                    for i in range(2):
                        ob = obs[i]
                        x2r = xtmp.tile([P, C], dt.float32, tag="x2t")
                        nc.sync.dma_start(
                            x2r[:], x2_dram[ob * P:(ob + 1) * P, :])
                        res = evict.tile([P, C], dt.float32, tag="res")
                        for j in range(2):
                            ncols = slice(j * 384, (j + 1) * 384)
                            tmp = evict.tile([P, 384], dt.float32, tag="tmp")
                            nc.vector.scalar_tensor_tensor(
                                tmp[:], pss[i][j][:, :384], 0.0,
                                bprr[:, ncols], op0=ALU.add, op1=ALU.add)
                            nc.vector.tensor_add(
                                res[:, ncols], tmp[:], x2r[:, ncols])
                        nc.sync.dma_start(
                            out[ob * P:(ob + 1) * P, :], res[:])

    return nc


_PROGRAM = None


def kernel(**inputs):
    global _PROGRAM
    _install_birpatch()
    _install_ntff_hook()
    from concourse.bass_utils import run_bass_kernel_spmd

    if _PROGRAM is None:
        _PROGRAM = _build_program()
    in_maps = _shard_inputs(inputs)
    res = run_bass_kernel_spmd(_PROGRAM, in_maps, list(range(N_CORES)))
    out = np.empty((B, T, C), dtype=np.float32)
    for i in range(N_CORES):
        b, g = i // 2, i % 2
        out[b, g * TOWN:(g + 1) * TOWN, :] = res.results[i]["out"]
    return out


def _shard_inputs(inputs):
    f32 = lambda a: np.ascontiguousarray(np.asarray(a, dtype=np.float32))
    x = f32(inputs["x"])
    w_attn = f32(inputs["w_attn"])
    b_attn = f32(inputs["b_attn"])
    w_o = f32(inputs["w_o"])
    shared = {
        "bo": f32(inputs["b_o"]),
        "ln1s": f32(inputs["ln1_s"]),
        "ln1b": f32(inputs["ln1_b"]),
        "ln2s": f32(inputs["ln2_s"]),
        "ln2b": f32(inputs["ln2_b"]),
        "wfc": f32(inputs["w_fc"]),
        "bfc": f32(inputs["b_fc"]),
        "wpr": f32(inputs["w_proj"]),
        "bpr": f32(inputs["b_proj"]),
    }
    maps = []
    for i in range(N_CORES):
        b, g = i // 2, i % 2
        hs = slice(g * HG * HD, (g + 1) * HG * HD)
        wq, wk, wv = (w_attn[:, k * C:][:, hs] for k in range(3))
        bq, bk, bv = (b_attn[k * C:][hs] for k in range(3))
        maps.append({
            "xb": x[b],
            "xown": np.ascontiguousarray(x[b, g * TOWN:(g + 1) * TOWN, :]),
            "wqkv": np.ascontiguousarray(
                np.concatenate([wq, wk, wv], axis=1)),
            "bqkv": np.ascontiguousarray(np.concatenate([bq, bk, bv])),
            "wo": np.ascontiguousarray(w_o[hs, :]),
            **shared,
        })
    return maps


# revision 11
# speedup vs baseline: 1.0798x; 1.0798x over previous
"""Bass/Trainium2 kernel for a GPT-2-style transformer block (B=4, T=2048,
C=768, H=12, causal attention + GELU MLP), SPMD across 8 NeuronCores.

Sharding: core i = (batch b = i//2, head-group g = i%2, 6 heads each).
Per core: LN1 + QKV over its batch's 2048 tokens (its 6 heads only),
causal flash-style attention (scores computed transposed [k, q], softmax
denominator via an appended ones-column in V, constant-shift-free exp —
scores are bounded ~|8| for LN'd inputs), attention out-projection
partial sums, pairwise ReduceScatter(add) over {2b, 2b+1} which also
splits the 2048 tokens in half, then residual + LN2 + MLP on the core's
1024 tokens.  All matmuls run in bf16 with fp32 PSUM accumulation.
"""

import sys
import types

import numpy as np
import orjson

B, T, C, H = 4, 2048, 768, 12
HD = C // H          # 64 head dim
HG = H // 2          # 6 heads per core
OC = 3 * HG * HD     # 1152 qkv cols per core
EPS = 1e-5
N_CORES = 8
P = 128
NT = T // P          # 16 token blocks per batch
NC6 = C // P         # 6 c-tiles
TOWN = T // 2        # 1024 own tokens after reduce-scatter
NOB = TOWN // P      # 8 own token blocks
FF = 4 * C           # 3072
NFF = FF // P        # 24
VW = HD + 1          # 65 V cols per head incl the ones column


# ---------------------------------------------------------------------------
# Toolchain patches for this container:
# 1. walrus accepts only ONE sync wait per instruction; Tile emits more.
#    Split extras onto NoOp instructions inserted before the owner (same
#    engine, program order — semantically identical).
# 2. antenv.axon_hooks is absent, so trace=True would fail; register the
#    NTFF hook module ourselves (used by test.py; harmless otherwise).
# ---------------------------------------------------------------------------

def _install_birpatch():
    import concourse.bass as bass

    if getattr(bass.Bass.to_json_bytes, "_multiwait_patched", False):
        return
    orig = bass.Bass.to_json_bytes

    def split_multi_waits(bir):
        ctr = 0
        for fn in bir.get("functions", []):
            for blk in fn.get("blocks", []):
                out = []
                for inst in blk.get("instructions", []):
                    si = inst.get("sync_info")
                    waits = (si or {}).get("on_wait", [])
                    if len(waits) > 1:
                        for w in waits[:-1]:
                            ctr += 1
                            out.append({
                                "debug": inst.get("debug", 0),
                                "engine": inst["engine"],
                                "ins": [],
                                "outs": [],
                                "name": f"waitsplit-{ctr}",
                                "opcode": "NoOp",
                                "sync_info": {"on_update": [],
                                              "on_wait": [w]},
                            })
                        si["on_wait"] = waits[-1:]
                    out.append(inst)
                blk["instructions"] = out
        return bir

    def patched(self):
        return orjson.dumps(split_multi_waits(orjson.loads(orig(self))))

    patched._multiwait_patched = True
    bass.Bass.to_json_bytes = patched


def _install_ntff_hook():
    if "antenv.axon_hooks" in sys.modules:
        return
    try:
        from trn_agent_boot.trn_boot import _ntff_profile_via_ctypes

        hook = _ntff_profile_via_ctypes("/opt/axon/libaxon_pjrt.so")
    except Exception:
        hook = None
    m = types.ModuleType("antenv.axon_hooks")
    m.get_axon_ntff_profile_hook = lambda: hook
    m.set_axon_ntff_profile_hook = lambda h: None
    sys.modules["antenv.axon_hooks"] = m


def _build_program():
    import concourse.bass as bass
    import concourse.mybir as mybir
    from concourse.masks import make_identity
    from concourse.tile import TileContext

    dt = mybir.dt
    AF = mybir.ActivationFunctionType
    ALU = mybir.AluOpType

    nc = bass.Bass("TRN2", target_bir_lowering=False, debug=False,
                   num_devices=N_CORES)

    # ---- I/O -----------------------------------------------------------
    xb = nc.dram_tensor("xb", [T, C], dt.float32, kind="ExternalInput")
    xown = nc.dram_tensor("xown", [TOWN, C], dt.float32, kind="ExternalInput")
    wqkv = nc.dram_tensor("wqkv", [C, OC], dt.float32, kind="ExternalInput")
    bqkv = nc.dram_tensor("bqkv", [OC], dt.float32, kind="ExternalInput")
    wo = nc.dram_tensor("wo", [HG * HD, C], dt.float32, kind="ExternalInput")
    bo = nc.dram_tensor("bo", [C], dt.float32, kind="ExternalInput")
    ln1s = nc.dram_tensor("ln1s", [C], dt.float32, kind="ExternalInput")
    ln1b = nc.dram_tensor("ln1b", [C], dt.float32, kind="ExternalInput")
    ln2s = nc.dram_tensor("ln2s", [C], dt.float32, kind="ExternalInput")
    ln2b = nc.dram_tensor("ln2b", [C], dt.float32, kind="ExternalInput")
    wfc = nc.dram_tensor("wfc", [C, FF], dt.float32, kind="ExternalInput")
    bfc = nc.dram_tensor("bfc", [FF], dt.float32, kind="ExternalInput")
    wpr = nc.dram_tensor("wpr", [FF, C], dt.float32, kind="ExternalInput")
    bpr = nc.dram_tensor("bpr", [C], dt.float32, kind="ExternalInput")
    out = nc.dram_tensor("out", [TOWN, C], dt.float32, kind="ExternalOutput")

    # internal DRAM
    po_dram = nc.dram_tensor("po_dram", [T, C], dt.float32)
    rs_dram = nc.dram_tensor("rs_dram", [TOWN, C], dt.float32)
    r_dram = nc.dram_tensor("r_dram", [HG * 4, 512], dt.float32)
    x2_dram = nc.dram_tensor("x2_dram", [TOWN, C], dt.float32)
    wfcb_dram = nc.dram_tensor("wfcb_dram", [C, FF], dt.bfloat16)
    wprb_dram = nc.dram_tensor("wprb_dram", [FF, C], dt.bfloat16)

    groups = [[0, 1], [2, 3], [4, 5], [6, 7]]

    with TileContext(nc) as tc:
        with (
            tc.tile_pool(name="const", bufs=1) as const,
            tc.tile_pool(name="persist", bufs=1) as persist,
            tc.tile_pool(name="wstage", bufs=2) as wstage,
            tc.tile_pool(name="wsbp", bufs=1) as wsbp,
            tc.tile_pool(name="rbp", bufs=2) as rbp,
            tc.tile_pool(name="rpool", bufs=2) as rpool,
            tc.tile_pool(name="psrb", bufs=1, space="PSUM") as psrb,
            tc.tile_pool(name="wstream", bufs=4) as wstream,
            tc.tile_pool(name="xtmp", bufs=2) as xtmp,
            tc.tile_pool(name="stat", bufs=4) as statp,
            tc.tile_pool(name="epool", bufs=3) as epool,
            tc.tile_pool(name="evict", bufs=2) as evict,
            tc.tile_pool(name="psmm", bufs=4, space="PSUM") as psmm,
            tc.tile_pool(name="psctx", bufs=2, space="PSUM") as psctx,
            tc.tile_pool(name="pstr", bufs=1, space="PSUM") as pstr,
        ):
            # ---- constants -------------------------------------------
            ident = const.tile([P, P], dt.bfloat16, tag="ident")
            make_identity(nc, ident)
            tri = const.tile([P, 896], dt.bfloat16, tag="tri")
            nc.gpsimd.memset(tri[:], 1.0)
            nc.gpsimd.affine_select(
                out=tri[:], in_=tri[:], compare_op=ALU.is_ge, fill=0.0,
                base=-384, pattern=[[1, 896]], channel_multiplier=-1)

            def rep_row(drt, tag):
                t = const.tile([P, C], dt.float32, tag=tag)
                nc.sync.dma_start(
                    t[:], drt.ap().unsqueeze(0).broadcast_to((P, C)))
                return t

            # early-phase / late-phase pairs share a slot
            s1r = rep_row(ln1s, "rrowA")
            b1r = rep_row(ln1b, "rrowB")
            s2r = rep_row(ln2s, "rrowC")
            b2r = rep_row(ln2b, "rrowD")
            bor = rep_row(bo, "rrowE")
            bprr = rep_row(bpr, "rrowF")
            bqkv9 = const.tile([P, OC // P], dt.float32, tag="bq9")
            nc.sync.dma_start(
                bqkv9[:], bqkv.ap().rearrange("(t p) -> p t", p=P))
            bfc24 = const.tile([P, NFF], dt.float32, tag="bf24")
            nc.sync.dma_start(
                bfc24[:], bfc.ap().rearrange("(t p) -> p t", p=P))
            epsb = const.tile([P, 1], dt.float32, tag="epsb")
            nc.gpsimd.memset(epsb[:], EPS)
            ones64 = const.tile([1, HD], dt.float32, tag="ones64")
            nc.gpsimd.memset(ones64[:], 1.0)
            bvr = const.tile([P, HG * HD], dt.float32, tag="bvr")
            nc.sync.dma_start(
                bvr[:],
                bqkv.ap()[2 * HG * HD:].unsqueeze(0)
                .broadcast_to((P, HG * HD)))

            # ---- resident weights: wqkv, wo (bf16) -------------------
            wqkv_b = [persist.tile([P, OC], dt.bfloat16, name=f"wqkv{c}", tag=f"wqkv{c}")
                      for c in range(NC6)]
            for c in range(NC6):
                wt = wstage.tile([P, 1536], dt.float32, tag="wst")
                nc.sync.dma_start(wt[:, :OC], wqkv[c * P:(c + 1) * P, :])
                nc.vector.tensor_copy(wqkv_b[c][:], wt[:, :OC])
            wo_b = [persist.tile([P, C], dt.bfloat16, name=f"wo{d}", tag=f"wo{d}")
                    for d in range(3)]
            for d in range(3):
                wt = wstage.tile([P, 1536], dt.float32, tag="wst")
                nc.sync.dma_start(wt[:, :C], wo[d * P:(d + 1) * P, :])
                nc.vector.tensor_copy(wo_b[d][:], wt[:, :C])

            # ---- pre-cast wfc / wpr to bf16 in DRAM (streamed later) --
            for c in range(NC6):
                for hh in range(2):
                    cols = slice(hh * 1536, (hh + 1) * 1536)
                    wt = wstage.tile([P, 1536], dt.float32, tag="wst")
                    nc.sync.dma_start(wt[:], wfc[c * P:(c + 1) * P, cols])
                    wb = wsbp.tile([P, 1536], dt.bfloat16, tag="wsb")
                    nc.vector.tensor_copy(wb[:], wt[:])
                    nc.sync.dma_start(
                        wfcb_dram[c * P:(c + 1) * P, cols], wb[:])
            for h in range(NFF):
                wt = wstage.tile([P, 1536], dt.float32, tag="wst")
                nc.sync.dma_start(wt[:, :C], wpr[h * P:(h + 1) * P, :])
                wb = wsbp.tile([P, 1536], dt.bfloat16, tag="wsb")
                nc.vector.tensor_copy(wb[:, :C], wt[:, :C])
                nc.sync.dma_start(wprb_dram[h * P:(h + 1) * P, :], wb[:, :C])

            # ---- LN1 + transpose -> xnT ------------------------------
            xnT = [[persist.tile([P, T // 2], dt.bfloat16, name=f"xnT{c}_{th}", tag=f"xnT{c}_{th}")
                    for th in range(2)] for c in range(NC6)]

            def layernorm_tile(xt, sr, br, dst_bf):
                st6 = statp.tile([P, 2, 6], dt.float32, tag="st6")
                st2 = statp.tile([P, 2], dt.float32, tag="st2")
                nc.vector.bn_stats(st6[:, 0, :], xt[:, 0:384])
                nc.vector.bn_stats(st6[:, 1, :], xt[:, 384:768])
                nc.vector.bn_aggr(st2[:], st6[:])
                sd = statp.tile([P, 1], dt.float32, tag="sd")
                nc.scalar.activation(sd[:], st2[:, 1:2], AF.Sqrt, bias=epsb[:])
                rs = statp.tile([P, 1], dt.float32, tag="rs")
                nc.vector.reciprocal(rs[:], sd[:])
                t1 = xtmp.tile([P, C], dt.float32, tag="t1")
                nc.vector.scalar_tensor_tensor(
                    t1[:], xt[:], st2[:, 0:1], sr[:],
                    op0=ALU.subtract, op1=ALU.mult)
                nc.vector.scalar_tensor_tensor(
                    dst_bf[:], t1[:], rs[:], br[:],
                    op0=ALU.mult, op1=ALU.add)

            for ti in range(NT):
                xt = xtmp.tile([P, C], dt.float32, tag="xt")
                nc.sync.dma_start(xt[:], xb[ti * P:(ti + 1) * P, :])
                xn = xtmp.tile([P, C], dt.bfloat16, tag="xn")
                layernorm_tile(xt, s1r, b1r, xn)
                th, tcol = ti // (NT // 2), (ti % (NT // 2)) * P
                for c in range(NC6):
                    pt = pstr.tile([P, P], dt.bfloat16, tag="ptr")
                    nc.tensor.transpose(
                        pt[:], xn[:, c * P:(c + 1) * P], ident[:])
                    nc.vector.tensor_copy(
                        xnT[c][th][:, tcol:tcol + P], pt[:])

            # ---- QKV: qkT [o, t] (Q tiles 0-2, K tiles 3-5) ----------
            qkT = [persist.tile([P, T], dt.bfloat16, name=f"qkT{o}", tag=f"qkT{o}")
                   for o in range(6)]
            vnat = [persist.tile([P, HG * VW], dt.bfloat16, name=f"vnat{tb}", tag=f"vnat{tb}")
                    for tb in range(NT)]
            for th in range(2):
                for tck in range(2):
                    cols = slice(tck * 512, (tck + 1) * 512)
                    gcol = th * 1024 + tck * 512
                    for o in range(6):
                        ps = psmm.tile([P, 512], dt.float32, tag="mm")
                        for c in range(NC6):
                            nc.tensor.matmul(
                                ps[:], wqkv_b[c][:, o * P:(o + 1) * P],
                                xnT[c][th][:, cols],
                                start=(c == 0), stop=(c == NC6 - 1))
                        nc.scalar.activation(
                            qkT[o][:, gcol:gcol + 512], ps[:], AF.Identity,
                            bias=bqkv9[:, o:o + 1])
            # V in natural layout [t, head*65] with a ones column per head
            for tb in range(NT):
                nc.gpsimd.memset(vnat[tb][:], 1.0)
                ps = psmm.tile([P, 512], dt.float32, tag="mm")
                th, tcol = tb // (NT // 2), (tb % (NT // 2)) * P
                for c in range(NC6):
                    nc.tensor.matmul(
                        ps[:, :HG * HD],
                        xnT[c][th][:, tcol:tcol + P],
                        wqkv_b[c][:, 2 * HG * HD:3 * HG * HD],
                        start=(c == 0), stop=(c == NC6 - 1))
                vv = vnat[tb].rearrange("p (h w) -> p h w", w=VW)
                nc.vector.scalar_tensor_tensor(
                    vv[:, :, 0:HD],
                    ps[:, :HG * HD].rearrange("p (h w) -> p h w", w=HD),
                    0.0,
                    bvr.rearrange("p (h w) -> p h w", w=HD),
                    op0=ALU.add, op1=ALU.add)

            # ---- causal attention (scores transposed [k, q]) ---------
            ctxT = [persist.tile([P, T], dt.bfloat16, name=f"ctxT{d}", tag=f"ctxT{d}")
                    for d in range(3)]
            for h in range(HG):
                ot = h // 2
                prow = (h % 2) * HD
                for qc in range(4):
                    qcols = slice(qc * 512, (qc + 1) * 512)
                    nkb = 4 * (qc + 1)
                    ctx = psctx.tile([VW, 512], dt.float32, tag="ctx")
                    for kb in range(nkb):
                        st = psmm.tile([P, 512], dt.float32, tag="mm")
                        nc.tensor.matmul(
                            st[:],
                            qkT[3 + ot][prow:prow + HD, kb * P:(kb + 1) * P],
                            qkT[ot][prow:prow + HD, qcols],
                            start=True, stop=True)
                        e = epool.tile([P, 512], dt.bfloat16, tag="e")
                        nc.scalar.activation(e[:], st[:], AF.Exp, scale=0.125)
                        if kb >= 4 * qc:
                            j0 = 384 + qc * 512 - kb * P
                            nc.vector.tensor_mul(
                                e[:], e[:], tri[:, j0:j0 + 512])
                        nc.tensor.matmul(
                            ctx[:], vnat[kb][:, h * VW:(h + 1) * VW], e[:],
                            start=(kb == 0), stop=(kb == nkb - 1))
                    r = rpool.tile([1, 512], dt.float32, tag="r")
                    nc.vector.reciprocal(r[:], ctx[HD:HD + 1, :])
                    rbps = psrb.tile([HD, 512], dt.float32, tag="rbps")
                    nc.tensor.matmul(rbps[:], ones64[:], r[:],
                                     start=True, stop=True)
                    rb = rbp.tile([HD, 512], dt.float32, tag="rb")
                    nc.scalar.activation(rb[:], rbps[:], AF.Copy)
                    nc.vector.tensor_mul(
                        ctxT[ot][prow:prow + HD, qcols], ctx[0:HD, :], rb[:])

            # ---- attention out-projection partials -> po_dram --------
            for tb in range(NT):
                po = evict.tile([P, C], dt.float32, tag="po")
                for nh in range(2):
                    ps = psmm.tile([P, 512], dt.float32, tag="mm")
                    for d in range(3):
                        nc.tensor.matmul(
                            ps[:, :384], ctxT[d][:, tb * P:(tb + 1) * P],
                            wo_b[d][:, nh * 384:(nh + 1) * 384],
                            start=(d == 0), stop=(d == 2))
                    nc.scalar.activation(
                        po[:, nh * 384:(nh + 1) * 384], ps[:, :384], AF.Copy)
                nc.sync.dma_start(po_dram[tb * P:(tb + 1) * P, :], po[:])

            # ---- pairwise reduce-scatter over {2b, 2b+1} -------------
            nc.gpsimd.collective_compute(
                "ReduceScatter", mybir.AluOpType.add,
                replica_groups=groups,
                ins=[po_dram[:]], outs=[rs_dram[:]])

            # ---- residual + LN2 + transpose --------------------------
            # xn2T reuses the (now dead) qkT slots
            xn2T = [persist.tile([P, TOWN], dt.bfloat16, name=f"xn2T{c}", tag=f"qkT{c}")
                    for c in range(NC6)]
            for ob in range(NOB):
                rt = xtmp.tile([P, C], dt.float32, tag="xt")
                nc.sync.dma_start(rt[:], rs_dram[ob * P:(ob + 1) * P, :])
                xo = xtmp.tile([P, C], dt.float32, tag="t1")
                nc.sync.dma_start(xo[:], xown[ob * P:(ob + 1) * P, :])
                nc.vector.tensor_add(rt[:], rt[:], bor[:])
                x2t = xtmp.tile([P, C], dt.float32, tag="x2t")
                nc.vector.tensor_add(x2t[:], rt[:], xo[:])
                nc.sync.dma_start(x2_dram[ob * P:(ob + 1) * P, :], x2t[:])
                xn2 = xtmp.tile([P, C], dt.bfloat16, tag="xn")
                layernorm_tile(x2t, s2r, b2r, xn2)
                for c in range(NC6):
                    pt = pstr.tile([P, P], dt.bfloat16, tag="ptr")
                    nc.tensor.transpose(
                        pt[:], xn2[:, c * P:(c + 1) * P], ident[:])
                    nc.vector.tensor_copy(
                        xn2T[c][:, ob * P:(ob + 1) * P], pt[:])

            # ---- MLP (chunks of 512 own tokens) ----------------------
            # hT chunk tiles: first 12 reuse dead xnT slots
            hT = []
            for o in range(NFF):
                if o < 12:
                    tg = f"xnT{o % 6}_{o // 6}"
                else:
                    tg = f"hT{o}"
                hT.append(persist.tile([P, 512], dt.bfloat16, name=f"hT{o}", tag=tg))

            for ck in range(2):
                cols = slice(ck * 512, (ck + 1) * 512)
                # fc + gelu -> hT
                for o in range(NFF):
                    w = wstream.tile([P, C], dt.bfloat16, name=f"wfcs_{ck}_{o}", tag="wfcs")
                    nc.gpsimd.dma_start(
                        w.rearrange("p (c o) -> p c o", o=P),
                        wfcb_dram.ap()[:, o * P:(o + 1) * P].rearrange(
                            "(c p) o -> p c o", p=P))
                    ps = psmm.tile([P, 512], dt.float32, tag="mm")
                    for c in range(NC6):
                        nc.tensor.matmul(
                            ps[:], w[:, c * P:(c + 1) * P], xn2T[c][:, cols],
                            start=(c == 0), stop=(c == NC6 - 1))
                    nc.scalar.activation(
                        hT[o][:], ps[:], AF.Gelu_apprx_tanh,
                        bias=bfc24[:, o:o + 1])
                # proj2 + bias + residual -> out, two (2 ob x 2 nh) rounds
                for rnd in range(2):
                    obs = [ck * 4 + rnd * 2, ck * 4 + rnd * 2 + 1]
                    pss = [[psmm.tile([P, 512], dt.float32, name=f"pp{ck}_{rnd}_{i}_{j}", tag="mm")
                            for j in range(2)] for i in range(2)]
                    for ht in range(NFF):
                        w = wstream.tile([P, C], dt.bfloat16, name=f"wprs_{ck}_{rnd}_{ht}", tag="wprs")
                        nc.gpsimd.dma_start(
                            w[:], wprb_dram[ht * P:(ht + 1) * P, :])
                        for i in range(2):
                            lcol = (obs[i] - ck * 4) * P
                            for j in range(2):
                                nc.tensor.matmul(
                                    pss[i][j][:, :384],
                                    hT[ht][:, lcol:lcol + P],
                                    w[:, j * 384:(j + 1) * 384],
                                    start=(ht == 0), stop=(ht == NFF - 1))
                    for i in range(2):
                        ob = obs[i]
                        x2r = xtmp.tile([P, C], dt.float32, tag="x2t")
                        nc.sync.dma_start(
                            x2r[:], x2_dram[ob * P:(ob + 1) * P, :])
                        res = evict.tile([P, C], dt.float32, tag="res")
                        for j in range(2):
                            ncols = slice(j * 384, (j + 1) * 384)
                            tmp = evict.tile([P, 384], dt.float32, tag="tmp")
                            nc.vector.scalar_tensor_tensor(
                                tmp[:], pss[i][j][:, :384], 0.0,
                                bprr[:, ncols], op0=ALU.add, op1=ALU.add)
                            nc.vector.tensor_add(
                                res[:, ncols], tmp[:], x2r[:, ncols])
                        nc.sync.dma_start(
                            out[ob * P:(ob + 1) * P, :], res[:])

    return nc


_PROGRAM = None


def kernel(**inputs):
    global _PROGRAM
    _install_birpatch()
    _install_ntff_hook()
    from concourse.bass_utils import run_bass_kernel_spmd

    if _PROGRAM is None:
        _PROGRAM = _build_program()
    in_maps = _shard_inputs(inputs)
    res = run_bass_kernel_spmd(_PROGRAM, in_maps, list(range(N_CORES)))
    out = np.empty((B, T, C), dtype=np.float32)
    for i in range(N_CORES):
        b, g = i // 2, i % 2
        out[b, g * TOWN:(g + 1) * TOWN, :] = res.results[i]["out"]
    return out


def _shard_inputs(inputs):
    f32 = lambda a: np.ascontiguousarray(np.asarray(a, dtype=np.float32))
    x = f32(inputs["x"])
    w_attn = f32(inputs["w_attn"])
    b_attn = f32(inputs["b_attn"])
    w_o = f32(inputs["w_o"])
    shared = {
        "bo": f32(inputs["b_o"]),
        "ln1s": f32(inputs["ln1_s"]),
        "ln1b": f32(inputs["ln1_b"]),
        "ln2s": f32(inputs["ln2_s"]),
        "ln2b": f32(inputs["ln2_b"]),
        "wfc": f32(inputs["w_fc"]),
        "bfc": f32(inputs["b_fc"]),
        "wpr": f32(inputs["w_proj"]),
        "bpr": f32(inputs["b_proj"]),
    }
    maps = []
    for i in range(N_CORES):
        b, g = i // 2, i % 2
        hs = slice(g * HG * HD, (g + 1) * HG * HD)
        wq, wk, wv = (w_attn[:, k * C:][:, hs] for k in range(3))
        bq, bk, bv = (b_attn[k * C:][hs] for k in range(3))
        maps.append({
            "xb": x[b],
            "xown": np.ascontiguousarray(x[b, g * TOWN:(g + 1) * TOWN, :]),
            "wqkv": np.ascontiguousarray(
                np.concatenate([wq, wk, wv], axis=1)),
            "bqkv": np.ascontiguousarray(np.concatenate([bq, bk, bv])),
            "wo": np.ascontiguousarray(w_o[hs, :]),
            **shared,
        })
    return maps
